# revision 1
# baseline (speedup 1.0000x reference)
"""3-layer GCN (PyG GCNConv semantics) on 8 Trainium2 NeuronCores.

Math (per layer, matching the reference exactly):
    y = x @ W
    deg[d] = (# edges into d) + 1,  dinv = deg^-1/2
    out[d] = dinv[d] * ( sum_{e: dst=d} dinv[src_e] * y[src_e] + dinv[d]*y[d] ) + b
The norm dinv[src]*dinv[dst] is separable: fold dinv[src] into a per-node
table  ytab = dinv * (x @ W)  and dinv[dst] into the per-edge mask weight.
The self-loop term is just an extra edge (d, d).

Distribution: nodes (dsts) sharded across 8 cores; each core owns a
contiguous 12544-padded shard.  Each layer:
  1. table phase: core computes ytab for its shard (x_fm slice @ W, scaled),
     writes to DRAM, AllGather -> full table replica per core.
  2. aggregation: dma_gather (int16 idx, 4 src-blocks of 25088 rows) streams
     per-edge table rows; per 128-edge chunk a selection mask
     (iota == dst_rel) * dinv[dst]  routes edges to dst columns; PE matmul
     msgs^T @ mask accumulates a [64, 128] feature-major psum per dst tile;
     psum tiles add into the x_fm accumulator.
All cores run ONE shared instruction schedule (envelope-padded chunk counts);
per-core structure lives in data tensors (gather idxs, dst_rel, edinv).
"""

import math
import os
import numpy as np

import concourse.bass as bass
import concourse.bacc as bacc
import concourse.mybir as mybir
import concourse.tile as tile
from concourse.bass_utils import run_bass_kernel_spmd

P = 128
H = 64
NCORES = 8
GROUP_TILES = 14          # dst tiles per gather call
MSG_BUFS = 2
F32 = mybir.dt.float32
F16 = mybir.dt.float16
HT = 2 * H     # padded table row width (256B in fp16)


def _round_up(a, b):
    return (a + b - 1) // b * b


# ----------------------------------------------------------------------------
# host-side schedule + per-core tensors
# ----------------------------------------------------------------------------
def _build_schedule(edge_index, N):
    src = np.asarray(edge_index[0], dtype=np.int64)
    dst = np.asarray(edge_index[1], dtype=np.int64)
    E = src.shape[0]

    shard_real = (N + NCORES - 1) // NCORES           # 12500
    shard_p = _round_up(shard_real, P)                # 12544
    k_sh = max(1, 32768 // shard_p)                   # shards per idx block
    block_rows = k_sh * shard_p                       # 25088
    nblocks = (NCORES * shard_p + block_rows - 1) // block_rows
    ntiles = shard_p // P                             # 98

    degree = np.bincount(dst, minlength=N).astype(np.int64)
    deg = degree.astype(np.float64) + 1.0
    dinv = (1.0 / np.sqrt(deg)).astype(np.float32)

    # degree-sorted relabeling per shard: tightens the cross-core chunk-count
    # envelope (tiles hold similar-degree nodes on every core)
    perms = []          # perms[c][sorted_pos] = original local id
    newpos = np.empty(N, np.int64)   # node -> sorted position within shard
    for c in range(NCORES):
        lo = c * shard_real
        hi = min(N, lo + shard_real)
        d_loc = degree[lo:hi]
        pc = np.argsort(-d_loc, kind="stable")
        perms.append(pc)
        inv = np.empty(hi - lo, np.int64)
        inv[pc] = np.arange(hi - lo)
        newpos[lo:hi] = inv

    src2 = src
    dst2 = dst

    core_of = dst2 // shard_real
    src_gid = (src2 // shard_real) * shard_p + newpos[src2]
    ldst = newpos[dst2]

    tile_id = ldst // P
    blk_id = src_gid // block_rows
    bucket = blk_id * ntiles + tile_id                # (s, t) bucket
    nbuck = nblocks * ntiles

    per_core = []
    counts = np.zeros((NCORES, nbuck), np.int64)
    for c in range(NCORES):
        m = core_of == c
        b_c = bucket[m]
        counts[c] = np.bincount(b_c, minlength=nbuck)
        per_core.append((b_c, src_gid[m], ldst[m], dst2[m]))

    cmax = counts.max(0)
    C_ts = (cmax + P - 1) // P                        # chunks per bucket
    S_ts = C_ts * P                                   # padded cells
    base = np.zeros(nbuck + 1, np.int64)
    np.cumsum(S_ts, out=base[1:])
    total_cells = int(base[-1])
    nchunks = total_cells // P

    core_tensors = []
    for c in range(NCORES):
        b_c, sg_c, ld_c, d_c = per_core[c]
        order = np.argsort(b_c, kind="stable")
        b_s = b_c[order]
        cnt = counts[c]
        starts = np.zeros(nbuck, np.int64)
        np.cumsum(cnt[:-1], out=starts[1:])
        rank = np.arange(b_s.shape[0], dtype=np.int64) - np.repeat(starts, cnt)
        pos = base[b_s] + rank

        idx_cells = np.full(total_cells, shard_real, np.int64)   # pad row
        dstrel_cells = np.full(total_cells, -1.0, np.float32)
        edinv_cells = np.zeros(total_cells, np.float32)
        idx_cells[pos] = sg_c[order] - blk_id_of(b_s, ntiles) * block_rows
        dstrel_cells[pos] = (ld_c[order] % P).astype(np.float32)
        edinv_cells[pos] = dinv[d_c[order]]

        assert idx_cells.max() < 32768 and idx_cells.min() >= 0
        idx16 = idx_cells.astype(np.int16).reshape(-1, 16).T      # [16, cols]
        idx_w = np.tile(idx16, (8, 1)).copy()                     # [128, cols]
        dst_rel = dstrel_cells.reshape(nchunks, P).T.copy()       # [128, nch]
        edinv_t = edinv_cells.reshape(nchunks, P).T.copy()

        # per-shard dinv column [128, ntiles] (pads -> 1.0), sorted order
        lo = c * shard_real
        hi = min(N, lo + shard_real)
        dvec = np.ones(shard_p, np.float32)
        dvec[: hi - lo] = dinv[lo:hi][perms[c]]
        dinv_col = dvec.reshape(ntiles, P).T.copy()

        # self chunks: one per tile, appended after gather chunks
        nreal = hi - lo
        sdst_tm = np.full((ntiles, P), -1.0, np.float32)
        sed_tm = np.zeros((ntiles, P), np.float32)
        valid = np.arange(shard_p) < nreal
        sdst_tm.reshape(-1)[valid] = (np.arange(shard_p) % P)[valid]
        sed_tm.reshape(-1)[valid] = dvec[valid]
        dst_rel = np.concatenate([dst_rel, sdst_tm.T], axis=1)
        edinv_t = np.concatenate([edinv_t, sed_tm.T], axis=1)

        core_tensors.append(
            dict(idx=idx_w, dst_rel=dst_rel, edinv=edinv_t, dinv_col=dinv_col,
                 perm=perms[c])
        )

    sched = dict(
        shard_real=shard_real,
        shard_p=shard_p,
        block_rows=block_rows,
        nblocks=nblocks,
        ntiles=ntiles,
        C_ts=C_ts.reshape(nblocks, ntiles),
        base=base.reshape(-1),
        total_cells=total_cells,
        nchunks=nchunks,
    )
    return sched, core_tensors


def blk_id_of(bucket, ntiles):
    return bucket // ntiles


# ----------------------------------------------------------------------------
# bass builder
# ----------------------------------------------------------------------------
def _build_bass(sched):
    shard_p = sched["shard_p"]
    block_rows = sched["block_rows"]
    nblocks = sched["nblocks"]
    ntiles = sched["ntiles"]
    C_ts = sched["C_ts"]
    base = sched["base"]
    total_cells = sched["total_cells"]
    nchunks = sched["nchunks"]
    table_rows = NCORES * shard_p

    TSIM = bool(int(os.environ.get("GNN_TSIM", "0")))
    nc = bacc.Bacc("TRN2", target_bir_lowering=False,
                   num_devices=1 if TSIM else NCORES,
                   dynamic_dma_scratch_size=int(os.environ.get("GNN_SCRATCH", "65536")))

    emb_in = nc.dram_tensor("emb_fm", [H, shard_p], F32, kind="ExternalInput")
    idx_in = nc.dram_tensor("idx", [P, total_cells // 16], mybir.dt.int16,
                            kind="ExternalInput")
    dst_rel_in = nc.dram_tensor("dst_rel", [P, nchunks + ntiles], F32, kind="ExternalInput")
    edinv_in = nc.dram_tensor("edinv", [P, nchunks + ntiles], F32, kind="ExternalInput")
    dinv_col_in = nc.dram_tensor("dinv_col", [P, ntiles], F32, kind="ExternalInput")
    iota_in = nc.dram_tensor("iota", [P, P], F16, kind="ExternalInput")
    w_ins = [nc.dram_tensor(f"W{l+1}", [H, H], F32, kind="ExternalInput")
             for l in range(3)]
    b_ins = [nc.dram_tensor(f"b{l+1}", [H, 1], F32, kind="ExternalInput")
             for l in range(3)]
    out_fm = nc.dram_tensor("out_fm", [H, shard_p], F32, kind="ExternalOutput")

    # partition tiles into groups; per (block, group) gather calls stay
    # under MAX_CALL_BLOCKS chunks
    MAX_CALL_BLOCKS = int(os.environ.get("GNN_MCB", "14"))
    tile_groups = []
    cur = [0, 0, 0]  # t0, t1, max-per-block cells
    t_ = 0
    percol = C_ts.max(axis=0)  # worst block per tile
    while t_ < ntiles:
        c = int(percol[t_])
        if cur[2] + c > MAX_CALL_BLOCKS and cur[2] > 0:
            tile_groups.append((cur[0], cur[1]))
            cur = [t_, t_, 0]
        cur[1] = t_ + 1
        cur[2] += c
        t_ += 1
    if cur[1] > cur[0]:
        tile_groups.append((cur[0], cur[1]))
    STAGE = os.environ.get("GNN_STAGE", "full")
    CUT = set(os.environ.get("GNN_CUT", "").split(","))
    NL = int(os.environ.get("GNN_NLAYERS", "3"))

    with tile.TileContext(nc) as tc:
        with (
            tc.tile_pool(name="persist", bufs=1) as persist,
            tc.tile_pool(name="msgs", bufs=MSG_BUFS) as msgs_pool,
            tc.tile_pool(name="masks", bufs=int(os.environ.get("GNN_MKB", "16"))) as mask_pool,
            tc.tile_pool(name="stg", bufs=3) as stg_pool,
            tc.tile_pool(name="ps_agg", bufs=int(os.environ.get("GNN_PSB", "4")), space="PSUM") as ps_agg,
            tc.tile_pool(name="ps_tb", bufs=2, space="PSUM") as ps_tb,
            tc.tile_pool(name="dram", bufs=1, space="DRAM") as dram,
        ):
            # ---- persistent SBUF ----
            x_fm = persist.tile([H, shard_p], F32)
            yshard = persist.tile([P, ntiles, HT], F16)
            nc.vector.memset(yshard[:], 0.0)
            idx_sb = persist.tile([P, total_cells // 16], mybir.dt.int16)
            dst_rel = persist.tile([P, nchunks + ntiles], F32)
            edinv = persist.tile([P, nchunks + ntiles], F32)
            dinv_col = persist.tile([P, ntiles], F32)
            iota_sb = persist.tile([P, P], F16)
            w_sb = [persist.tile([H, H], F32, name=f"w{l}") for l in range(3)]
            b_sb = [persist.tile([H, 1], F32, name=f"b{l}") for l in range(3)]

            nc.sync.dma_start(out=x_fm[:], in_=emb_in[:])
            nc.sync.dma_start(out=idx_sb[:], in_=idx_in[:])
            nc.sync.dma_start(out=dst_rel[:], in_=dst_rel_in[:])
            nc.sync.dma_start(out=edinv[:], in_=edinv_in[:])
            nc.sync.dma_start(out=dinv_col[:], in_=dinv_col_in[:])
            nc.sync.dma_start(out=iota_sb[:], in_=iota_in[:])
            for l in range(3):
                nc.sync.dma_start(out=w_sb[l][:], in_=w_ins[l][:])
                nc.sync.dma_start(out=b_sb[l][:], in_=b_ins[l][:])

            ag_in = [dram.tile([shard_p, HT], F16, name=f"agin{i}") for i in range(3)]
            tables = [dram.tile([table_rows, HT], F16, addr_space="Shared",
                                name=f"table{i}") for i in range(3)]

            MASK_G = 12

            def emit_masks(ch0, G, tag):
                mb = mask_pool.tile([P, MASK_G * P], F16, tag=tag)
                a = dst_rel[:, ch0:ch0 + G]
                dr_b = bass.AP(a.tensor, a.offset, [list(a.ap[0]), list(a.ap[1]), [0, P]])
                e = edinv[:, ch0:ch0 + G]
                ed_b = bass.AP(e.tensor, e.offset, [list(e.ap[0]), list(e.ap[1]), [0, P]])
                i = iota_sb[:, :]
                io_b = bass.AP(i.tensor, i.offset, [list(i.ap[0]), [0, G], list(i.ap[1])])
                mbv = mb[:, 0:G * P].rearrange("p (g q) -> p g q", q=P)
                nc.vector.tensor_tensor(out=mbv, in0=io_b, in1=dr_b,
                                        op=mybir.AluOpType.is_equal)
                nc.vector.tensor_tensor(out=mbv, in0=mbv, in1=ed_b,
                                        op=mybir.AluOpType.mult)
                return mb

            for l in range(NL):
                tb = tables[l]
                gi = ag_in[l]
                # ---- table phase ----
                for t in range(0 if STAGE == "agonly" else ntiles):
                    pt = ps_tb.tile([P, H], F32, space="PSUM", tag="pt")
                    nc.tensor.matmul(
                        out=pt[:],
                        lhsT=x_fm[:, t * P:(t + 1) * P],
                        rhs=w_sb[l][:],
                        start=True, stop=True,
                    )
                    nc.vector.tensor_scalar_mul(
                        yshard[:, t, 0:H], pt[:], dinv_col[:, t:t + 1])
                # batched shard -> DRAM (14 tiles per DMA)
                for t0 in range(0, ntiles, 14):
                    t1 = min(ntiles, t0 + 14)
                    nc.sync.dma_start(
                        out=gi[t0 * P:t1 * P, :].rearrange(
                            "(a p) h -> p a h", p=P),
                        in_=yshard[:, t0:t1, :],
                    )
                if STAGE == "table":
                    continue
                if TSIM:
                    nc.sync.dma_start(out=tb[0:shard_p, :], in_=gi[:])
                elif True:
                    nc.gpsimd.collective_compute(
                        "AllGather",
                        mybir.AluOpType.bypass,
                        replica_groups=[list(range(NCORES))],
                        ins=[gi[:].opt()],
                        outs=[tb[: NCORES * shard_p, :].opt()],
                    )
                if STAGE == "ag":
                    dbg = stg_pool.tile([H, H], F32, tag="dbg")
                    nc.sync.dma_start(out=dbg[:], in_=tb[l * H:(l + 1) * H, :])
                    nc.sync.dma_start(out=out_fm[:, l * H:(l + 1) * H], in_=dbg[:])
                    continue

                # ---- aggregation (group-major, segments inner) ----
                CUT = set(os.environ.get("GNN_CUT", "").split(","))
                const_mask = mask_pool.tile([P, P], F16, tag="cmask")
                nc.vector.tensor_scalar(
                    const_mask[:], iota_sb[:], dst_rel[:, 0:1], edinv[:, 0:1],
                    mybir.AluOpType.is_equal, mybir.AluOpType.mult,
                )

                def mk_mask(ch):
                    if "mask" in CUT:
                        return const_mask
                    mask = mask_pool.tile([P, P], F16, tag="mask")
                    nc.vector.tensor_scalar(
                        mask[:], iota_sb[:],
                        dst_rel[:, ch:ch + 1],
                        edinv[:, ch:ch + 1],
                        mybir.AluOpType.is_equal,
                        mybir.AluOpType.mult,
                    )
                    return mask

                for (t0, t1) in tile_groups:
                    bufs = []
                    for s in range(nblocks):
                        cell0 = int(base[s * ntiles + t0])
                        cell1 = int(base[s * ntiles + t1])
                        n_sg = cell1 - cell0
                        if n_sg == 0:
                            bufs.append((None, 0))
                            continue
                        buf = msgs_pool.tile([P, n_sg // P, HT], F16,
                                             tag=f"msgs{s}", bufs=int(os.environ.get("GNN_MGB", "2")))
                        if "gather" in CUT:
                            nc.vector.memset(buf[0:1, 0, :], 0.0)
                            bufs.append((buf, cell0))
                            continue
                        nc.gpsimd.dma_gather(
                            buf[:],
                            tb[s * block_rows:min((s + 1) * block_rows,
                                                   table_rows), :],
                            idx_sb[:, cell0 // 16: cell1 // 16],
                            n_sg, n_sg, HT,
                            single_packet=False,
                        )
                        bufs.append((buf, cell0))
                    for t in range(t0, t1):
                        psum = ps_agg.tile([H, P], F32, space="PSUM",
                                           tag="pagg")
                        mi = 0
                        for s in range(nblocks):
                            C = int(C_ts[s][t])
                            if C == 0:
                                continue
                            buf, cell0 = bufs[s]
                            cbase = int(base[s * ntiles + t])
                            for c in range(C):
                                ch = (cbase + c * P) // P
                                blk = (cbase - cell0) // P + c
                                mask = mk_mask(ch)
                                if "mm" not in CUT:
                                    nc.tensor.matmul(
                                        out=psum[:],
                                        lhsT=buf[:, blk, 0:H],
                                        rhs=mask[:],
                                        start=(mi == 0), stop=False,
                                    )
                                    mi += 1
                        mask = mk_mask(nchunks + t)
                        nc.tensor.matmul(
                            out=psum[:],
                            lhsT=yshard[:, t, 0:H],
                            rhs=mask[:],
                            start=(mi == 0), stop=True,
                        )
                        # psum -> x_fm with bias folded (per-partition b)
                        nc.vector.tensor_scalar(
                            x_fm[:, t * P:(t + 1) * P], psum[:],
                            b_sb[l][:], None, mybir.AluOpType.add,
                        )
            nc.sync.dma_start(out=out_fm[:], in_=x_fm[:])

    nc.compile()
    return nc


_CACHE = {}


def kernel(embeddings, edge_index, W1, b1, W2, b2, W3, b3):
    embeddings = np.ascontiguousarray(np.asarray(embeddings, dtype=np.float32))
    edge_index = np.asarray(edge_index)
    N = embeddings.shape[0]

    sched, core_tensors = _build_schedule(edge_index, N)
    shard_real, shard_p = sched["shard_real"], sched["shard_p"]

    key = (N, edge_index.shape[1], sched["total_cells"], os.environ.get("GNN_STAGE"), os.environ.get("GNN_NLAYERS"), os.environ.get("GNN_SCRATCH"), os.environ.get("GNN_TSIM"))
    if key not in _CACHE:
        _CACHE[key] = _build_bass(sched)
    nc = _CACHE[key]

    iota = np.tile(np.arange(P, dtype=np.float16), (P, 1)).copy()
    ws = [np.asarray(W, np.float32) for W in (W1, W2, W3)]
    bs = [np.asarray(b, np.float32).reshape(H, 1) for b in (b1, b2, b3)]

    in_maps = []
    for c in range(NCORES):
        lo = c * shard_real
        hi = min(N, lo + shard_real)
        ct = core_tensors[c]
        emb_fm = np.zeros((H, shard_p), np.float32)
        emb_fm[:, : hi - lo] = embeddings[lo:hi][ct["perm"]].T
        m = dict(
            emb_fm=emb_fm,
            idx=ct["idx"],
            dst_rel=ct["dst_rel"],
            edinv=ct["edinv"],
            dinv_col=ct["dinv_col"],
            iota=iota,
            W1=ws[0], W2=ws[1], W3=ws[2],
            b1=bs[0], b2=bs[1], b3=bs[2],
        )
        in_maps.append(m)

    res = run_bass_kernel_spmd(nc, in_maps, core_ids=list(range(NCORES)))
    out = np.empty((N, H), np.float32)
    for c in range(NCORES):
        lo = c * shard_real
        hi = min(N, lo + shard_real)
        out[lo + core_tensors[c]["perm"]] = res.results[c]["out_fm"].T[: hi - lo]
    return out


def prepare(embeddings, edge_index, W1, b1, W2, b2, W3, b3):
    """Build (nc, in_maps) once for repeated benchmarking."""
    embeddings = np.ascontiguousarray(np.asarray(embeddings, dtype=np.float32))
    edge_index = np.asarray(edge_index)
    N = embeddings.shape[0]
    sched, core_tensors = _build_schedule(edge_index, N)
    shard_real, shard_p = sched["shard_real"], sched["shard_p"]
    key = (N, edge_index.shape[1], sched["total_cells"], os.environ.get("GNN_STAGE"),
           os.environ.get("GNN_NLAYERS"), os.environ.get("GNN_SCRATCH"), os.environ.get("GNN_TSIM"), os.environ.get("GNN_CUT"))
    if key not in _CACHE:
        _CACHE[key] = _build_bass(sched)
    nc = _CACHE[key]
    iota = np.tile(np.arange(P, dtype=np.float16), (P, 1)).copy()
    ws = [np.asarray(W, np.float32) for W in (W1, W2, W3)]
    bs = [np.asarray(b, np.float32).reshape(H, 1) for b in (b1, b2, b3)]
    in_maps = []
    for c in range(NCORES):
        lo = c * shard_real
        hi = min(N, lo + shard_real)
        ct = core_tensors[c]
        emb_fm = np.zeros((H, shard_p), np.float32)
        emb_fm[:, : hi - lo] = embeddings[lo:hi][ct["perm"]].T
        in_maps.append(dict(
            emb_fm=emb_fm, idx=ct["idx"], dst_rel=ct["dst_rel"],
            edinv=ct["edinv"], dinv_col=ct["dinv_col"], iota=iota,
            W1=ws[0], W2=ws[1], W3=ws[2], b1=bs[0], b2=bs[1], b3=bs[2],
        ))
    return nc, in_maps, sched

